# revision 5
# baseline (speedup 1.0000x reference)
"""MoE ConditionalFeedForward (SwiGLU, top-2 of 8 experts) on 8 TRN2 NeuronCores.

Strategy: expert-parallel. Core e owns expert e's weights (w1/w2/w3 slices).
The host routes tokens: for each expert, gather the tokens assigned to it
(deduped per token, padded to CAP), each core computes
    y = (silu(x @ w1[e].T) * (x @ w3[e].T)) @ w2[e].T
densely for its gathered tokens, and the host scatters rows back into the
[T, A, D] output (duplicate (t,e) assignments computed once, written twice).

Per-core kernel layout (all host-pretransposed so every DMA is contiguous):
  xt  [128, 8*CAP]    xt[p, k*CAP+j] = x_g[j, k*128+p]     (tokens, transposed)
  w13 [11, 128, 4096] pair j = i-tiles (2j, 2j+1); per i-tile w1 cols then w3
  w2s [11, 128, 2048] w2s[j,p,h*1024+d] = w2[e][d, (2j+h)*128+p]
  yt  [128, 8*CAP]    f32, yt[p, k*CAP+j] = y_g[j, k*128+p] (output, transposed)

DMA plan (both HW rings, w13 strictly ahead of w2):
  Sync ring  (upfront): x k0-slice, w13 even halves (pair0 split in quarters),
                        then w2 even pairs.
  Scalar ring: x rest + w13 odd halves 0-1 upfront; one issue per phase-A
               i-tile after that (w13 odd 2-10, then w2 odd pairs) so the
               issues never block the silu stream but stay well ahead of use.
All weight tiles are SBUF-resident (no pool cycling): issues never wait on
consumption, so the queues stream back-to-back at full rate.

Phase A (per i-tile it of 22): h1T/h3T [128(i), CAP] = sum_k wT @ x tiles in
PSUM, then hT = silu(h1)*h3 into SBUF. Phase B: single k-major loop --
yT[d-tile k] [128, CAP] accumulated over all 22 i-tiles in PSUM, drained and
DMA'd (rings alternating) per k while k+1 accumulates.
"""

import numpy as np
from contextlib import ExitStack

import concourse.bass as bass
import concourse.bacc as bacc
import concourse.mybir as mybir
import concourse.tile as tile
from concourse.bass_utils import run_bass_kernel_spmd

E, I, D = 8, 2816, 1024
N_CORES = 8
NI, ND = I // 128, D // 128  # 22, 8

# storage dtype for weights/activations on-device: "bfloat16" (half HBM
# traffic, full PE rate) or "float32" (matmuls run as float32r, 2 cyc/row)
DT_NAME = "bfloat16"

_PROG_CACHE: dict = {}


def _build_program(cap: int, dt_name: str):
    DT = mybir.dt.float32r if dt_name == "float32" else getattr(mybir.dt, dt_name)
    f32 = mybir.dt.float32
    NP = NI // 2  # 11 (w13/w2 DMAs batched as i-tile pairs for >=0.5MB transfers)
    nc = bacc.Bacc("TRN2", target_bir_lowering=False, debug=False)
    xt = nc.dram_tensor("xt", [128, ND * cap], DT, kind="ExternalInput").ap()
    w13 = nc.dram_tensor("w13", [NP, 128, 4 * D], DT, kind="ExternalInput").ap()
    w2s = nc.dram_tensor("w2s", [NP, 128, 2 * D], DT, kind="ExternalInput").ap()
    yt = nc.dram_tensor("yt", [128, ND * cap], f32, kind="ExternalOutput").ap()
    warm_out = nc.dram_tensor("warm_out", [128, 16], f32, kind="ExternalOutput").ap()

    with tile.TileContext(nc) as tc, ExitStack() as ctx:
        warmp = ctx.enter_context(tc.tile_pool(name="warm", bufs=1))
        xp = ctx.enter_context(tc.tile_pool(name="x", bufs=1))
        w13p = ctx.enter_context(tc.tile_pool(name="w13", bufs=1))
        hp = ctx.enter_context(tc.tile_pool(name="h", bufs=NI))
        silp = ctx.enter_context(tc.tile_pool(name="sil", bufs=3))
        w2p = ctx.enter_context(tc.tile_pool(name="w2", bufs=1))
        yp = ctx.enter_context(tc.tile_pool(name="y", bufs=1))

        # All weight tiles resident in SBUF (160KB/partition total, fits in
        # the ~208KB budget) so every DMA can be enqueued without waiting on
        # consumers and the two HW rings stream back-to-back.
        xsb = xp.tile([128, ND * cap], DT)
        w13ts = [
            w13p.tile([128, 4 * D], DT, tag=f"w13_{j}", name=f"w13_{j}")
            for j in range(NP)
        ]
        w2ts = [
            w2p.tile([128, 2 * D], DT, tag=f"w2_{j}", name=f"w2_{j}")
            for j in range(NP)
        ]

        # Sync ring, in queue order: x k0-slice (needed first), w13 even
        # halves -- pair 0 as two quarters so the very first i-tile's w1
        # lands earlier -- then ALL w2 pairs. The ring is FIFO, so w2 only
        # streams after every even w13 half is delivered: no explicit pacing
        # needed, and the scalar ring keeps bandwidth for the odd halves
        # during phase A. Sync has no compute, so when the issues hit the
        # ring's in-flight descriptor window and block, nothing is delayed.
        nc.sync.dma_start(xsb[:, 0:cap], xt[:, 0:cap])
        nc.sync.dma_start(w13ts[0][:, 0:D], w13[0][:, 0:D])
        nc.sync.dma_start(w13ts[0][:, D : 2 * D], w13[0][:, D : 2 * D])
        for j in range(1, NP):
            nc.sync.dma_start(w13ts[j][:, 0 : 2 * D], w13[j][:, 0 : 2 * D])
        for j in range(NP):
            nc.sync.dma_start(w2ts[j][:], w2s[j])

        # Scalar ring: rest of x (needed by the first i-tile's k>=1 steps),
        # then all odd w13 halves. Exactly 12 issues -- within the ring's
        # in-flight window, so none of them can block and stall the silu
        # stream behind them (an in-loop issue that waits for queue room
        # blocks every later silu: ~19us stall observed).
        nc.scalar.dma_start(xsb[:, cap:], xt[:, cap:])
        for j in range(NP):
            nc.scalar.dma_start(w13ts[j][:, 2 * D :], w13[j][:, 2 * D :])

        # PE warmup: matmuls on a zeroed tile, no DMA dependency. Sized to
        # keep the PE busy from ~7.6us until the first real weights land
        # (~13.5us: NEFF preamble + DMA-ring ramp): the HAM clock-gate needs
        # ~3.4us of SUSTAINED busy to release, and any idle gap before the
        # flip restarts the clock ramp (measured: a 4us gap left the first
        # ~30 real matmuls at 1.2GHz). 8 cold matmuls (427ns) + 11 warm
        # (213ns) ~= 5.8us of bridge.
        with tc.tile_pool(name="warmps", bufs=1, space="PSUM") as warmps:
            wtile = warmp.tile([128, 640], DT)
            nc.gpsimd.memset(wtile[:], 0.0)
            wps = warmps.tile([128, 512], f32)
            n_warm = 19
            for i in range(n_warm):
                nc.tensor.matmul(
                    wps[:],
                    wtile[:, 0:128],
                    wtile[:, 128:640],
                    start=(i == 0),
                    stop=(i == n_warm - 1),
                )
            wsc = warmp.tile([128, 16], f32)
            nc.vector.tensor_copy(wsc[:], wps[:, 0:16])
            nc.gpsimd.dma_start(warm_out[:], wsc[:])

        hts = []
        with tc.tile_pool(name="hps", bufs=3, space="PSUM") as hps:
            for it in range(NI):
                j, half = it // 2, it % 2
                wt = w13ts[j]
                base = half * 2 * D
                h1 = hps.tile([128, cap], f32, tag="h1", name="h1")
                h3 = hps.tile([128, cap], f32, tag="h3", name="h3")
                for k in range(ND):
                    nc.tensor.matmul(
                        h1[:],
                        wt[:, base + k * 128 : base + (k + 1) * 128],
                        xsb[:, k * cap : (k + 1) * cap],
                        start=(k == 0),
                        stop=(k == ND - 1),
                    )
                for k in range(ND):
                    nc.tensor.matmul(
                        h3[:],
                        wt[:, base + D + k * 128 : base + D + (k + 1) * 128],
                        xsb[:, k * cap : (k + 1) * cap],
                        start=(k == 0),
                        stop=(k == ND - 1),
                    )
                sil = silp.tile([128, cap], f32)
                nc.scalar.activation(
                    sil[:], h1[:], mybir.ActivationFunctionType.Silu
                )
                ht = hp.tile([128, cap], DT)
                nc.vector.tensor_mul(ht[:], sil[:], h3[:])
                hts.append(ht)

        # Phase B: one k-major pass; yT[d-tile k] accumulates over all 22
        # i-tiles (all w2 pairs are in flight well before phase B starts),
        # then drains to SBUF + DMA (alternating rings) while k+1 runs.
        with tc.tile_pool(name="yps", bufs=1, space="PSUM") as yps:
            ypt = [
                yps.tile([128, cap], f32, tag=f"yps_{k}", name=f"yps_{k}")
                for k in range(ND)
            ]
            ysb = yp.tile([128, ND * cap], f32)
            for k in range(ND):
                for it in range(NI):
                    nc.tensor.matmul(
                        ypt[k][:],
                        w2ts[it // 2][:, (it % 2) * D + k * 128 : (it % 2) * D + (k + 1) * 128],
                        hts[it][:],
                        start=(it == 0),
                        stop=(it == NI - 1),
                    )
                dst = ysb[:, k * cap : (k + 1) * cap]
                nc.vector.tensor_copy(dst, ypt[k][:])
                eng = nc.sync if k % 2 == 0 else nc.scalar
                eng.dma_start(yt[:, k * cap : (k + 1) * cap], dst)

    nc.compile()
    return nc


def _get_program(cap: int, dt_name: str):
    key = (cap, dt_name)
    if key not in _PROG_CACHE:
        _PROG_CACHE[key] = _build_program(cap, dt_name)
    return _PROG_CACHE[key]


def _np_dt(dt_name: str):
    if dt_name == "float32":
        return np.float32
    import ml_dtypes

    return ml_dtypes.bfloat16


def _prep_weights(w1, w3, w2, dt_name):
    """Per-expert pretransposed/tiled weight arrays (see module docstring)."""
    npdt = _np_dt(dt_name)
    w13_all, w2s_all = [], []
    for e in range(E):
        # [I, D] -> [it, c, k, p] -> [it, p, k, c] -> [it, 128, 1024]
        a1 = w1[e].reshape(NI, 128, ND, 128).transpose(0, 3, 2, 1).reshape(NI, 128, D)
        a3 = w3[e].reshape(NI, 128, ND, 128).transpose(0, 3, 2, 1).reshape(NI, 128, D)
        # pairs of i-tiles: [11, 128, 4096] = [w1|w3] for it=2j then it=2j+1
        a13 = np.concatenate([a1, a3], axis=2).reshape(NI // 2, 2, 128, 2 * D)
        w13_all.append(
            np.ascontiguousarray(a13.transpose(0, 2, 1, 3)).reshape(
                NI // 2, 128, 4 * D
            ).astype(npdt)
        )
        # w2[e] [D, I] -> T [I, D] -> [22, 128, 1024] -> pairs [11, 128, 2048]
        a2 = w2[e].T.reshape(NI // 2, 2, 128, D)
        w2s_all.append(
            np.ascontiguousarray(a2.transpose(0, 2, 1, 3)).reshape(
                NI // 2, 128, 2 * D
            ).astype(npdt)
        )
    return w13_all, w2s_all


def kernel(x, w1, w2, w3, expert_indices, _trace=False):
    x = np.asarray(x, dtype=np.float32)
    w1 = np.asarray(w1, dtype=np.float32)
    w2 = np.asarray(w2, dtype=np.float32)
    w3 = np.asarray(w3, dtype=np.float32)
    idx = np.asarray(expert_indices).astype(np.int64)
    T, A = idx.shape
    npdt = _np_dt(DT_NAME)

    # Per expert: unique token list (a token picking the same expert in both
    # slots is computed once) + the flat output positions mapped to each row.
    tok_lists, pos_lists, inv_lists = [], [], []
    flat = idx.ravel()  # position p = t*A + a -> expert id
    for e in range(E):
        pos = np.nonzero(flat == e)[0]
        toks, inv = np.unique(pos // A, return_inverse=True)
        tok_lists.append(toks)
        pos_lists.append(pos)
        inv_lists.append(inv)
    counts = np.array([len(t) for t in tok_lists])

    w13_all, w2s_all = _prep_weights(w1, w3, w2, DT_NAME)

    out = np.empty((T * A, D), dtype=np.float32)
    done = np.zeros(E, dtype=np.int64)
    last_res = None
    while (counts - done).max() > 0:
        remaining = counts - done
        cap = min(512, max(32, int(-(-remaining.max() // 16)) * 16))
        nc = _get_program(cap, DT_NAME)
        in_maps = []
        round_rows = []  # per-core (token rows processed this round)
        for e in range(E):
            n = int(min(remaining[e], cap))
            toks = tok_lists[e][done[e] : done[e] + n]
            round_rows.append((int(done[e]), n))
            xg = np.zeros((cap, D), dtype=np.float32)
            xg[:n] = x[toks]
            # [cap, D] -> T [D, cap] -> [k, 128, cap] -> [128, k, cap]
            xt_host = np.ascontiguousarray(
                xg.T.reshape(ND, 128, cap).transpose(1, 0, 2)
            ).reshape(128, ND * cap).astype(npdt)
            in_maps.append({"xt": xt_host, "w13": w13_all[e], "w2s": w2s_all[e]})
            done[e] += n
        last_res = run_bass_kernel_spmd(
            nc, in_maps, core_ids=list(range(N_CORES)), trace=_trace
        )
        for e in range(E):
            lo, n = round_rows[e]
            if n == 0:
                continue
            # yt [128, 8*cap] -> [p, k, j] -> y[j, k*128+p]
            ye = (
                last_res.results[e]["yt"]
                .reshape(128, ND, cap)
                .transpose(2, 1, 0)
                .reshape(cap, D)
            )
            inv = inv_lists[e]
            pos = pos_lists[e]
            sel = (inv >= lo) & (inv < lo + n)
            out[pos[sel]] = ye[inv[sel] - lo]

    result = out.reshape(T, A, D)
    if _trace:
        return result, last_res
    return result


# revision 7
# speedup vs baseline: 1.1552x; 1.1552x over previous
"""MoE ConditionalFeedForward (SwiGLU, top-2 of 8 experts) on 8 TRN2 NeuronCores.

Strategy: expert-parallel. Core e owns expert e's weights (w1/w2/w3 slices).
The host routes tokens: for each expert, gather the tokens assigned to it
(deduped per token, padded to CAP), each core computes
    y = (silu(x @ w1[e].T) * (x @ w3[e].T)) @ w2[e].T
densely for its gathered tokens, and the host scatters rows back into the
[T, A, D] output (duplicate (t,e) assignments computed once, written twice).

Per-core kernel layout. All dram tensors are [128, N] with the partition dim
first and exactly the SBUF layout, so every DMA is a plain 2D row copy:
  xt   [128, 8*CAP]     xt[p, k*CAP+j] = x_g[j, k*128+p]
  w13e [128, 11*2048]   even i-tiles:  cols it/2*2048 + [w1-tile | w3-tile]
  w13o [128, 11*2048]   odd i-tiles
  w2t  [128, 22*1024]   w2t[p, it*1024+d] = w2[e][d, it*128+p]
  yt   [128, 8*CAP]     f32, yt[p, k*CAP+j] = y_g[j, k*128+p]

DMA plan. Constraints learned from traces: (1) each HW ring (Sync, Scalar)
blocks the *issuing engine* once ~8 descriptors are in flight, so the Scalar
ring -- which also runs the silu stream -- must never have enough issues
queued to block; (2) the rings share ~400GB/s and are FIFO, so consumption
order == issue order is the only pacing needed.
  Sync upfront:  x k0-slice, w13e pair0 in quarters, w13e in 1MB batches,
                 then w2 pairs 0-5 in 1MB batches (10 issues; the last ones
                 briefly wait for queue room, which is free on Sync).
  Scalar:        x rest + w13o in 1MB batches (7 issues, under the window),
                 then w2 pairs 6-10 as 2 batches issued mid-phase-A when the
                 queue has provably drained.

Phase A (per i-tile it of 22): h1T/h3T [128(i), CAP] = sum_k wT @ x tiles in
PSUM, then hT = silu(h1)*h3 into SBUF. Phase B: single k-major loop --
yT[d-tile k] [128, CAP] accumulated over all 22 i-tiles in PSUM, drained and
DMA'd (rings alternating) per k while k+1 accumulates.
"""

import numpy as np
from contextlib import ExitStack

import concourse.bass as bass
import concourse.bacc as bacc
import concourse.mybir as mybir
import concourse.tile as tile
from concourse.bass_utils import run_bass_kernel_spmd

E, I, D = 8, 2816, 1024
N_CORES = 8
NI, ND = I // 128, D // 128  # 22, 8
NP = NI // 2  # 11

DT_NAME = "bfloat16"

_PROG_CACHE: dict = {}


def _build_program(cap: int, dt_name: str):
    DT = mybir.dt.float32r if dt_name == "float32" else getattr(mybir.dt, dt_name)
    f32 = mybir.dt.float32
    W13C = NP * 2 * D  # 22528 cols per half tensor
    W2C = NI * D       # 22528
    nc = bacc.Bacc("TRN2", target_bir_lowering=False, debug=False)
    xt = nc.dram_tensor("xt", [128, ND * cap], DT, kind="ExternalInput").ap()
    w13e = nc.dram_tensor("w13e", [128, W13C], DT, kind="ExternalInput").ap()
    w13o = nc.dram_tensor("w13o", [128, W13C], DT, kind="ExternalInput").ap()
    w2t = nc.dram_tensor("w2t", [128, W2C], DT, kind="ExternalInput").ap()
    yt = nc.dram_tensor("yt", [128, ND * cap], f32, kind="ExternalOutput").ap()
    warm_out = nc.dram_tensor("warm_out", [128, 16], f32, kind="ExternalOutput").ap()

    with tile.TileContext(nc) as tc, ExitStack() as ctx:
        warmp = ctx.enter_context(tc.tile_pool(name="warm", bufs=1))
        xp = ctx.enter_context(tc.tile_pool(name="x", bufs=1))
        wp = ctx.enter_context(tc.tile_pool(name="w", bufs=1))
        hp = ctx.enter_context(tc.tile_pool(name="h", bufs=NI))
        silp = ctx.enter_context(tc.tile_pool(name="sil", bufs=3))
        yp = ctx.enter_context(tc.tile_pool(name="y", bufs=1))

        xsb = xp.tile([128, ND * cap], DT)
        w13e_sb = wp.tile([128, W13C], DT, tag="w13e", name="w13e_sb")
        w13o_sb = wp.tile([128, W13C], DT, tag="w13o", name="w13o_sb")
        w2_sb = wp.tile([128, W2C], DT, tag="w2", name="w2_sb")

        # Sync ring (no compute behind it; blocking on queue room is free)
        nc.sync.dma_start(xsb[:, 0:cap], xt[:, 0:cap])
        nc.sync.dma_start(w13e_sb[:, 0:D], w13e[:, 0:D])
        nc.sync.dma_start(w13e_sb[:, D : 2 * D], w13e[:, D : 2 * D])
        for j in range(1, NP, 2):
            hi = min(j + 2, NP)
            nc.sync.dma_start(
                w13e_sb[:, j * 2 * D : hi * 2 * D], w13e[:, j * 2 * D : hi * 2 * D]
            )
        for j in range(0, 6, 2):  # w2 pairs 0-5, 1MB batches
            nc.sync.dma_start(
                w2_sb[:, j * 2 * D : (j + 2) * 2 * D],
                w2t[:, j * 2 * D : (j + 2) * 2 * D],
            )

        # Scalar ring: exactly 7 upfront issues (x rest, w13o pair 0, then
        # 1MB batches) -- under the in-flight window so the silu stream
        # behind them can never be blocked.
        nc.scalar.dma_start(xsb[:, cap:], xt[:, cap:])
        nc.scalar.dma_start(w13o_sb[:, 0 : 2 * D], w13o[:, 0 : 2 * D])
        for j in range(1, NP, 2):
            hi = min(j + 2, NP)
            nc.scalar.dma_start(
                w13o_sb[:, j * 2 * D : hi * 2 * D], w13o[:, j * 2 * D : hi * 2 * D]
            )
        # w2 pairs 6-10 issued mid-phase-A (see loop below)
        scalar_q = {
            4: (w2_sb[:, 6 * 2 * D : 8 * 2 * D], w2t[:, 6 * 2 * D : 8 * 2 * D]),
            8: (w2_sb[:, 8 * 2 * D : W2C], w2t[:, 8 * 2 * D : W2C]),
        }

        # PE warmup bridge: keep the PE busy from ~7.6us until the first
        # real weights land (~13.5us = NEFF preamble + DMA-ring ramp). The
        # HAM clock-gate needs ~3.4us of SUSTAINED busy to release, and an
        # idle gap before the flip restarts the ramp (measured: a 4us gap
        # left ~30 real matmuls at 1.2GHz).
        with tc.tile_pool(name="warmps", bufs=1, space="PSUM") as warmps:
            wtile = warmp.tile([128, 640], DT)
            nc.gpsimd.memset(wtile[:], 0.0)
            wps = warmps.tile([128, 512], f32)
            n_warm = 19
            for i in range(n_warm):
                nc.tensor.matmul(
                    wps[:],
                    wtile[:, 0:128],
                    wtile[:, 128:640],
                    start=(i == 0),
                    stop=(i == n_warm - 1),
                )
            wsc = warmp.tile([128, 16], f32)
            nc.vector.tensor_copy(wsc[:], wps[:, 0:16])
            nc.gpsimd.dma_start(warm_out[:], wsc[:])

        hts = []
        with tc.tile_pool(name="hps", bufs=3, space="PSUM") as hps:
            for it in range(NI):
                if it in scalar_q:
                    nc.scalar.dma_start(*scalar_q[it])
                j = it // 2
                wt = w13e_sb if it % 2 == 0 else w13o_sb
                base = j * 2 * D
                h1 = hps.tile([128, cap], f32, tag="h1", name="h1")
                h3 = hps.tile([128, cap], f32, tag="h3", name="h3")
                for k in range(ND):
                    nc.tensor.matmul(
                        h1[:],
                        wt[:, base + k * 128 : base + (k + 1) * 128],
                        xsb[:, k * cap : (k + 1) * cap],
                        start=(k == 0),
                        stop=(k == ND - 1),
                    )
                for k in range(ND):
                    nc.tensor.matmul(
                        h3[:],
                        wt[:, base + D + k * 128 : base + D + (k + 1) * 128],
                        xsb[:, k * cap : (k + 1) * cap],
                        start=(k == 0),
                        stop=(k == ND - 1),
                    )
                sil = silp.tile([128, cap], f32, tag="sil", name="sil")
                nc.scalar.activation(
                    sil[:], h1[:], mybir.ActivationFunctionType.Silu
                )
                ht = hp.tile([128, cap], DT, tag="ht", name="ht")
                nc.vector.tensor_mul(ht[:], sil[:], h3[:])
                hts.append(ht)

        # Phase B: one k-major pass; yT[d-tile k] accumulates over all 22
        # i-tiles, then drains to SBUF + DMA (alternating rings) while k+1
        # runs.
        with tc.tile_pool(name="yps", bufs=1, space="PSUM") as yps:
            ypt = [
                yps.tile([128, cap], f32, tag=f"yps_{k}", name=f"yps_{k}")
                for k in range(ND)
            ]
            ysb = yp.tile([128, ND * cap], f32)
            for k in range(ND):
                for it in range(NI):
                    nc.tensor.matmul(
                        ypt[k][:],
                        w2_sb[:, it * D + k * 128 : it * D + (k + 1) * 128],
                        hts[it][:],
                        start=(it == 0),
                        stop=(it == NI - 1),
                    )
                dst = ysb[:, k * cap : (k + 1) * cap]
                nc.vector.tensor_copy(dst, ypt[k][:])
                eng = nc.sync if k % 2 == 0 else nc.scalar
                eng.dma_start(yt[:, k * cap : (k + 1) * cap], dst)

    nc.compile()
    return nc


def _get_program(cap: int, dt_name: str):
    key = (cap, dt_name)
    if key not in _PROG_CACHE:
        _PROG_CACHE[key] = _build_program(cap, dt_name)
    return _PROG_CACHE[key]


def _np_dt(dt_name: str):
    if dt_name == "float32":
        return np.float32
    import ml_dtypes

    return ml_dtypes.bfloat16


def _prep_weights(w1, w3, w2, dt_name):
    """Per-expert pretransposed weight arrays in dram layout (see docstring)."""
    npdt = _np_dt(dt_name)
    w13e_all, w13o_all, w2_all = [], [], []
    for e in range(E):
        # a1[it, p, k*128+c] = w1[e][it*128+c, k*128+p]
        a1 = w1[e].reshape(NI, 128, ND, 128).transpose(0, 3, 2, 1).reshape(NI, 128, D)
        a3 = w3[e].reshape(NI, 128, ND, 128).transpose(0, 3, 2, 1).reshape(NI, 128, D)
        a13 = np.concatenate([a1, a3], axis=2)  # [22, 128, 2048]
        w13e_all.append(
            np.ascontiguousarray(a13[0::2].transpose(1, 0, 2)).reshape(
                128, NP * 2 * D
            ).astype(npdt)
        )
        w13o_all.append(
            np.ascontiguousarray(a13[1::2].transpose(1, 0, 2)).reshape(
                128, NP * 2 * D
            ).astype(npdt)
        )
        # a2[it, p, d] = w2[e][d, it*128+p]
        a2 = w2[e].T.reshape(NI, 128, D)
        w2_all.append(
            np.ascontiguousarray(a2.transpose(1, 0, 2)).reshape(128, NI * D).astype(
                npdt
            )
        )
    return w13e_all, w13o_all, w2_all


def kernel(x, w1, w2, w3, expert_indices, _trace=False):
    x = np.asarray(x, dtype=np.float32)
    w1 = np.asarray(w1, dtype=np.float32)
    w2 = np.asarray(w2, dtype=np.float32)
    w3 = np.asarray(w3, dtype=np.float32)
    idx = np.asarray(expert_indices).astype(np.int64)
    T, A = idx.shape
    npdt = _np_dt(DT_NAME)

    # Per expert: unique token list (a token picking the same expert in both
    # slots is computed once) + flat output positions mapped to each row.
    tok_lists, pos_lists, inv_lists = [], [], []
    flat = idx.ravel()
    for e in range(E):
        pos = np.nonzero(flat == e)[0]
        toks, inv = np.unique(pos // A, return_inverse=True)
        tok_lists.append(toks)
        pos_lists.append(pos)
        inv_lists.append(inv)
    counts = np.array([len(t) for t in tok_lists])

    w13e_all, w13o_all, w2_all = _prep_weights(w1, w3, w2, DT_NAME)

    out = np.empty((T * A, D), dtype=np.float32)
    done = np.zeros(E, dtype=np.int64)
    last_res = None
    while (counts - done).max() > 0:
        remaining = counts - done
        cap = min(512, max(32, int(-(-remaining.max() // 16)) * 16))
        nc = _get_program(cap, DT_NAME)
        in_maps = []
        round_rows = []
        for e in range(E):
            n = int(min(remaining[e], cap))
            toks = tok_lists[e][done[e] : done[e] + n]
            round_rows.append((int(done[e]), n))
            xg = np.zeros((cap, D), dtype=np.float32)
            xg[:n] = x[toks]
            xt_host = np.ascontiguousarray(
                xg.T.reshape(ND, 128, cap).transpose(1, 0, 2)
            ).reshape(128, ND * cap).astype(npdt)
            in_maps.append(
                {
                    "xt": xt_host,
                    "w13e": w13e_all[e],
                    "w13o": w13o_all[e],
                    "w2t": w2_all[e],
                }
            )
            done[e] += n
        last_res = run_bass_kernel_spmd(
            nc, in_maps, core_ids=list(range(N_CORES)), trace=_trace
        )
        for e in range(E):
            lo, n = round_rows[e]
            if n == 0:
                continue
            ye = (
                last_res.results[e]["yt"]
                .reshape(128, ND, cap)
                .transpose(2, 1, 0)
                .reshape(cap, D)
            )
            inv = inv_lists[e]
            pos = pos_lists[e]
            sel = (inv >= lo) & (inv < lo + n)
            out[pos[sel]] = ye[inv[sel] - lo]

    result = out.reshape(T, A, D)
    if _trace:
        return result, last_res
    return result


# revision 10
# speedup vs baseline: 1.2742x; 1.1030x over previous
"""MoE ConditionalFeedForward (SwiGLU, top-2 of 8 experts) on 8 TRN2 NeuronCores.

Strategy: expert-parallel. Core e owns expert e's weights (w1/w2/w3 slices).
The host routes tokens: for each expert, gather the tokens assigned to it
(deduped per token, padded to CAP), each core computes
    y = (silu(x @ w1[e].T) * (x @ w3[e].T)) @ w2[e].T
densely for its gathered tokens, and the host scatters rows back into the
[T, A, D] output (duplicate (t,e) assignments computed once, written twice).

Per-core kernel layout. All dram tensors are [128, N] with the partition dim
first and exactly the SBUF layout, so every DMA is a plain 2D row copy:
  xt   [128, 8*CAP]     xt[p, k*CAP+j] = x_g[j, k*128+p]
  w13e [128, 11*2048]   even i-tiles:  cols it/2*2048 + [w1-tile | w3-tile]
  w13o [128, 11*2048]   odd i-tiles
  w2t  [128, 22*1024]   w2t[p, it*1024+d] = w2[e][d, it*128+p]
  yt   [128, 8*CAP]     f32, yt[p, k*CAP+j] = y_g[j, k*128+p]

DMA plan. Constraints learned from traces: (1) each HW ring (Sync, Scalar)
blocks the *issuing engine* once ~8 descriptors are in flight, so the Scalar
ring -- which also runs the silu stream -- must never have enough issues
queued to block; (2) the rings share ~400GB/s and are FIFO, so consumption
order == issue order is the only pacing needed.
  Sync upfront:  x k0-slice, w13e pair0 in quarters, w13e in 1MB batches,
                 then w2 pairs 0-5 in 1MB batches (10 issues; the last ones
                 briefly wait for queue room, which is free on Sync).
  Scalar:        x rest + w13o in 1MB batches (7 issues, under the window),
                 then w2 pairs 6-10 as 2 batches issued mid-phase-A when the
                 queue has provably drained.

Phase A (per i-tile it of 22): h1T/h3T [128(i), CAP] = sum_k wT @ x tiles in
PSUM, then hT = silu(h1)*h3 into SBUF. Phase B: single k-major loop --
yT[d-tile k] [128, CAP] accumulated over all 22 i-tiles in PSUM, drained and
DMA'd (rings alternating) per k while k+1 accumulates.
"""

import numpy as np
from contextlib import ExitStack

import concourse.bass as bass
import concourse.bacc as bacc
import concourse.mybir as mybir
import concourse.tile as tile
from concourse.bass_utils import run_bass_kernel_spmd

E, I, D = 8, 2816, 1024
N_CORES = 8
NI, ND = I // 128, D // 128  # 22, 8
NP = NI // 2  # 11

DT_NAME = "bfloat16"

_PROG_CACHE: dict = {}


def _build_program(cap: int, dt_name: str):
    DT = mybir.dt.float32r if dt_name == "float32" else getattr(mybir.dt, dt_name)
    f32 = mybir.dt.float32
    W13C = NP * 2 * D  # 22528 cols per half tensor
    W2C = NI * D       # 22528
    nc = bacc.Bacc("TRN2", target_bir_lowering=False, debug=False)
    xt = nc.dram_tensor("xt", [128, ND * cap], DT, kind="ExternalInput").ap()
    w13e = nc.dram_tensor("w13e", [128, W13C], DT, kind="ExternalInput").ap()
    w13o = nc.dram_tensor("w13o", [128, W13C], DT, kind="ExternalInput").ap()
    w2t = nc.dram_tensor("w2t", [128, W2C], DT, kind="ExternalInput").ap()
    yt = nc.dram_tensor("yt", [128, ND * cap], f32, kind="ExternalOutput").ap()
    warm_out = nc.dram_tensor("warm_out", [128, 16], f32, kind="ExternalOutput").ap()

    with tile.TileContext(nc) as tc, ExitStack() as ctx:
        warmp = ctx.enter_context(tc.tile_pool(name="warm", bufs=1))
        xp = ctx.enter_context(tc.tile_pool(name="x", bufs=1))
        wp = ctx.enter_context(tc.tile_pool(name="w", bufs=1))
        hp = ctx.enter_context(tc.tile_pool(name="h", bufs=NI))
        silp = ctx.enter_context(tc.tile_pool(name="sil", bufs=3))
        yp = ctx.enter_context(tc.tile_pool(name="y", bufs=1))

        xsb = xp.tile([128, ND * cap], DT)
        w13e_sb = wp.tile([128, W13C], DT, tag="w13e", name="w13e_sb")
        w13o_sb = wp.tile([128, W13C], DT, tag="w13o", name="w13o_sb")
        w2_sb = wp.tile([128, W2C], DT, tag="w2", name="w2_sb")

        # Sync ring (no compute behind it; blocking on queue room is free):
        # x k0-slice, even w13 halves -- pair 0 quartered and pairs 1-2 as
        # singles for fine-grained early completions, 1MB batches after --
        # then w2 pairs 0-5.
        def colspan(sb, dr, j0, j1, unit):
            return (sb[:, j0 * unit : j1 * unit], dr[:, j0 * unit : j1 * unit])

        nc.sync.dma_start(xsb[:, 0:cap], xt[:, 0:cap])
        nc.sync.dma_start(w13e_sb[:, 0:D], w13e[:, 0:D])
        nc.sync.dma_start(w13e_sb[:, D : 2 * D], w13e[:, D : 2 * D])
        for j0, j1 in ((1, 2), (2, 3), (3, 5), (5, 7), (7, 9), (9, 11)):
            nc.sync.dma_start(*colspan(w13e_sb, w13e, j0, j1, 2 * D))
        for j0, j1 in ((0, 2), (2, 4), (4, 6)):
            nc.sync.dma_start(*colspan(w2_sb, w2t, j0, j1, 2 * D))

        # Scalar ring also runs the silu stream, and the HW ring only keeps
        # ~2-3 descriptors in flight before the ISSUE instruction itself
        # blocks (issue #n waits for #n-2's transfer to complete), which
        # would stall every silu behind it. So: 3 issues upfront, the rest
        # paced through the phase-A loop at i-tiles where the guard is
        # already satisfied (each issue lands >=2 transfers after the one it
        # waits on, and >=4us before the weights are consumed).
        nc.scalar.dma_start(xsb[:, cap:], xt[:, cap:])
        nc.scalar.dma_start(w13o_sb[:, 0 : 2 * D], w13o[:, 0 : 2 * D])
        nc.scalar.dma_start(*colspan(w13o_sb, w13o, 1, 3, 2 * D))
        scalar_q = {
            1: colspan(w13o_sb, w13o, 3, 5, 2 * D),
            3: colspan(w13o_sb, w13o, 5, 7, 2 * D),
            6: colspan(w13o_sb, w13o, 7, 9, 2 * D),
            9: colspan(w13o_sb, w13o, 9, 11, 2 * D),
            12: colspan(w2_sb, w2t, 6, 8, 2 * D),
            15: colspan(w2_sb, w2t, 8, 11, 2 * D),
        }

        # PE warmup bridge: keep the PE busy from ~7.6us until the first
        # real weights land (~13.5us = NEFF preamble + DMA-ring ramp). The
        # HAM clock-gate needs ~3.4us of SUSTAINED busy to release, and an
        # idle gap before the flip restarts the ramp (measured: a 4us gap
        # left ~30 real matmuls at 1.2GHz).
        # Pre-load the silu activation table: the (auto-inserted, 1.3us)
        # ACT_TABLE_LOAD precedes the first ACTIVATE in the Scalar stream;
        # a dummy activation here runs it during the DMA window instead of
        # right before silu0 (where it delayed phase A by ~2us).
        with tc.tile_pool(name="warmps", bufs=1, space="PSUM") as warmps:
            wtile = warmp.tile([128, 640], DT)
            nc.gpsimd.memset(wtile[:], 0.0)
            tdum = warmp.tile([128, 16], f32)
            nc.scalar.activation(
                tdum[:], wtile[:, 0:16], mybir.ActivationFunctionType.Silu
            )
            wps = warmps.tile([128, 512], f32)
            n_warm = 19
            for i in range(n_warm):
                nc.tensor.matmul(
                    wps[:],
                    wtile[:, 0:128],
                    wtile[:, 128:640],
                    start=(i == 0),
                    stop=(i == n_warm - 1),
                )
            # Anchor the warmup against dead-code elimination: copy a sliver
            # of the PSUM result (vector is idle here); the DMA of it goes
            # out on Sync at the very END of the program so no compute
            # engine ever waits on it.
            wsc = warmp.tile([128, 16], f32)
            nc.vector.tensor_copy(wsc[:], wps[:, 0:16])

        hts = []
        with tc.tile_pool(name="hps", bufs=3, space="PSUM") as hps:
            for it in range(NI):
                if it in scalar_q:
                    nc.scalar.dma_start(*scalar_q[it])
                j = it // 2
                wt = w13e_sb if it % 2 == 0 else w13o_sb
                base = j * 2 * D
                h1 = hps.tile([128, cap], f32, tag="h1", name="h1")
                h3 = hps.tile([128, cap], f32, tag="h3", name="h3")
                for k in range(ND):
                    nc.tensor.matmul(
                        h1[:],
                        wt[:, base + k * 128 : base + (k + 1) * 128],
                        xsb[:, k * cap : (k + 1) * cap],
                        start=(k == 0),
                        stop=(k == ND - 1),
                    )
                for k in range(ND):
                    nc.tensor.matmul(
                        h3[:],
                        wt[:, base + D + k * 128 : base + D + (k + 1) * 128],
                        xsb[:, k * cap : (k + 1) * cap],
                        start=(k == 0),
                        stop=(k == ND - 1),
                    )
                sil = silp.tile([128, cap], f32, tag="sil", name="sil")
                nc.scalar.activation(
                    sil[:], h1[:], mybir.ActivationFunctionType.Silu
                )
                ht = hp.tile([128, cap], DT, tag="ht", name="ht")
                nc.vector.tensor_mul(ht[:], sil[:], h3[:])
                hts.append(ht)

        # Phase B: one k-major pass; yT[d-tile k] accumulates over all 22
        # i-tiles, then drains to SBUF + DMA (alternating rings) while k+1
        # runs.
        with tc.tile_pool(name="yps", bufs=1, space="PSUM") as yps:
            ypt = [
                yps.tile([128, cap], f32, tag=f"yps_{k}", name=f"yps_{k}")
                for k in range(ND)
            ]
            ysb = yp.tile([128, ND * cap], f32)
            for k in range(ND):
                for it in range(NI):
                    nc.tensor.matmul(
                        ypt[k][:],
                        w2_sb[:, it * D + k * 128 : it * D + (k + 1) * 128],
                        hts[it][:],
                        start=(it == 0),
                        stop=(it == NI - 1),
                    )
                dst = ysb[:, k * cap : (k + 1) * cap]
                nc.vector.tensor_copy(dst, ypt[k][:])
                eng = nc.sync if k % 2 == 0 else nc.scalar
                eng.dma_start(yt[:, k * cap : (k + 1) * cap], dst)
        nc.sync.dma_start(warm_out[:], wsc[:])

    nc.compile()
    return nc


def _get_program(cap: int, dt_name: str):
    key = (cap, dt_name)
    if key not in _PROG_CACHE:
        _PROG_CACHE[key] = _build_program(cap, dt_name)
    return _PROG_CACHE[key]


def _np_dt(dt_name: str):
    if dt_name == "float32":
        return np.float32
    import ml_dtypes

    return ml_dtypes.bfloat16


def _prep_weights(w1, w3, w2, dt_name):
    """Per-expert pretransposed weight arrays in dram layout (see docstring)."""
    npdt = _np_dt(dt_name)
    w13e_all, w13o_all, w2_all = [], [], []
    for e in range(E):
        # a1[it, p, k*128+c] = w1[e][it*128+c, k*128+p]
        a1 = w1[e].reshape(NI, 128, ND, 128).transpose(0, 3, 2, 1).reshape(NI, 128, D)
        a3 = w3[e].reshape(NI, 128, ND, 128).transpose(0, 3, 2, 1).reshape(NI, 128, D)
        a13 = np.concatenate([a1, a3], axis=2)  # [22, 128, 2048]
        w13e_all.append(
            np.ascontiguousarray(a13[0::2].transpose(1, 0, 2)).reshape(
                128, NP * 2 * D
            ).astype(npdt)
        )
        w13o_all.append(
            np.ascontiguousarray(a13[1::2].transpose(1, 0, 2)).reshape(
                128, NP * 2 * D
            ).astype(npdt)
        )
        # a2[it, p, d] = w2[e][d, it*128+p]
        a2 = w2[e].T.reshape(NI, 128, D)
        w2_all.append(
            np.ascontiguousarray(a2.transpose(1, 0, 2)).reshape(128, NI * D).astype(
                npdt
            )
        )
    return w13e_all, w13o_all, w2_all


def kernel(x, w1, w2, w3, expert_indices, _trace=False):
    x = np.asarray(x, dtype=np.float32)
    w1 = np.asarray(w1, dtype=np.float32)
    w2 = np.asarray(w2, dtype=np.float32)
    w3 = np.asarray(w3, dtype=np.float32)
    idx = np.asarray(expert_indices).astype(np.int64)
    T, A = idx.shape
    npdt = _np_dt(DT_NAME)

    # Per expert: unique token list (a token picking the same expert in both
    # slots is computed once) + flat output positions mapped to each row.
    tok_lists, pos_lists, inv_lists = [], [], []
    flat = idx.ravel()
    for e in range(E):
        pos = np.nonzero(flat == e)[0]
        toks, inv = np.unique(pos // A, return_inverse=True)
        tok_lists.append(toks)
        pos_lists.append(pos)
        inv_lists.append(inv)
    counts = np.array([len(t) for t in tok_lists])

    w13e_all, w13o_all, w2_all = _prep_weights(w1, w3, w2, DT_NAME)

    out = np.empty((T * A, D), dtype=np.float32)
    done = np.zeros(E, dtype=np.int64)
    last_res = None
    while (counts - done).max() > 0:
        remaining = counts - done
        cap = min(512, max(32, int(-(-remaining.max() // 16)) * 16))
        nc = _get_program(cap, DT_NAME)
        in_maps = []
        round_rows = []
        for e in range(E):
            n = int(min(remaining[e], cap))
            toks = tok_lists[e][done[e] : done[e] + n]
            round_rows.append((int(done[e]), n))
            xg = np.zeros((cap, D), dtype=np.float32)
            xg[:n] = x[toks]
            xt_host = np.ascontiguousarray(
                xg.T.reshape(ND, 128, cap).transpose(1, 0, 2)
            ).reshape(128, ND * cap).astype(npdt)
            in_maps.append(
                {
                    "xt": xt_host,
                    "w13e": w13e_all[e],
                    "w13o": w13o_all[e],
                    "w2t": w2_all[e],
                }
            )
            done[e] += n
        last_res = run_bass_kernel_spmd(
            nc, in_maps, core_ids=list(range(N_CORES)), trace=_trace
        )
        for e in range(E):
            lo, n = round_rows[e]
            if n == 0:
                continue
            ye = (
                last_res.results[e]["yt"]
                .reshape(128, ND, cap)
                .transpose(2, 1, 0)
                .reshape(cap, D)
            )
            inv = inv_lists[e]
            pos = pos_lists[e]
            sel = (inv >= lo) & (inv < lo + n)
            out[pos[sel]] = ye[inv[sel] - lo]

    result = out.reshape(T, A, D)
    if _trace:
        return result, last_res
    return result
